# revision 7
# baseline (speedup 1.0000x reference)
"""Multi-head attention (RoPE + causal mask) Trainium2 kernel, 8-core SPMD.

Sharding: 8 cores = 2 batches x 4 head-groups (4 heads of dk=128 each).
Each core computes q/k/v projections for its head-group, attention, and a
partial output projection; the host sums the 4 head-group partials per batch.

Per-core device program (Bass/Tile):
  - qT, kT computed transposed [dk, S] with RoPE fused at PSUM eviction
    (rotate-half via a +-1 permutation matmul on the PE), spilled to DRAM.
  - v computed natural [S, dv-group], resident in SBUF.
  - scores pass1 [s_q part, s_k free]: causal-mask add + row-max (DVE) and
    exp row-sum (ACT, fused accumulate); per-row softmax bias
    b = max + ln(sum)/scale broadcast across partitions via GpSimd.
  - scores pass2 transposed [s_k part, s_q free]: P^T = exp(scores^T - b)
    written straight to SBUF (no PE transposes of probabilities).
  - AV on PE: aoT[dv, s_q] += V^T_tile @ P^T_tile; O-projection accumulates
    the 4 heads in PSUM; y tiles DMA'd out.
  - matmul inputs viewed as float32r (full-rate fp32 path, fp32 PSUM accum).
"""

import numpy as np

import concourse.bacc as bacc
import concourse.mybir as mybir
from concourse.tile import TileContext
from concourse.masks import make_identity
from concourse.bass_utils import run_bass_kernel_spmd

F32 = mybir.dt.float32
F32R = mybir.dt.float32r
AX = mybir.AxisListType
OP = mybir.AluOpType
ACTF = mybir.ActivationFunctionType

B, S, D, H = 2, 2048, 2048, 16
DK = 128
NH = 4                      # heads per core
DH = NH * DK                # head-group width
N_CORES = 8
NEG_BIG = -1.0e9


def _r(ap):
    return ap.bitcast(F32R)


def build_nc(causal=True, S=S, DM=D, NH=NH, use_pbcast=True, use_affsel=True, phase2=True, p2_stage=9):
    DH_ = NH * DK
    n_dc = DM // DK
    n_sc = S // 512
    scale_c = 1.0 / float(np.sqrt(DK))
    inv_scale = float(np.sqrt(DK))

    nc = bacc.Bacc("TRN2", target_bir_lowering=False, debug=False,
                   enable_asserts=False, num_devices=N_CORES)

    xT = nc.dram_tensor("xT", (DM, S), F32, kind="ExternalInput").ap()
    wq = nc.dram_tensor("wq", (DM, DH_), F32, kind="ExternalInput").ap()
    wk = nc.dram_tensor("wk", (DM, DH_), F32, kind="ExternalInput").ap()
    wv = nc.dram_tensor("wv", (DM, DH_), F32, kind="ExternalInput").ap()
    wo = nc.dram_tensor("wo", (DH_, DM), F32, kind="ExternalInput").ap()
    bqc = nc.dram_tensor("bqc", (DK, NH), F32, kind="ExternalInput").ap()
    bkc = nc.dram_tensor("bkc", (DK, NH), F32, kind="ExternalInput").ap()
    bvr = nc.dram_tensor("bvr", (1, DH_), F32, kind="ExternalInput").ap()
    cosT = nc.dram_tensor("cosT", (DK, S), F32, kind="ExternalInput").ap()
    sinT = nc.dram_tensor("sinT", (DK, S), F32, kind="ExternalInput").ap()
    mb = nc.dram_tensor("mb", (4, DK, 512), F32, kind="ExternalInput").ap()
    y = nc.dram_tensor("y", (S, DM), F32, kind="ExternalOutput").ap()

    with TileContext(nc) as tc:
        with tc.tile_pool(name="const", bufs=1) as cpool, \
             tc.tile_pool(name="dram", bufs=1, space="DRAM") as dpool, \
             tc.tile_pool(name="vres", bufs=1) as vpool, \
             tc.tile_pool(name="psum", bufs=8, space="PSUM") as pp:

            ident = cpool.tile([128, 128], F32, name="ident")
            make_identity(nc, ident)
            # rotate-half matrix: rotm[d, m] = -1 if d==m+64, +1 if d==m-64
            rotm = cpool.tile([128, 128], F32, name="rotm")
            nc.gpsimd.memset(rotm, 0.0)
            nc.gpsimd.affine_select(
                out=rotm, in_=rotm, compare_op=OP.not_equal, fill=-1.0,
                base=-64, pattern=[[-1, 128]], channel_multiplier=1)
            nc.gpsimd.affine_select(
                out=rotm, in_=rotm, compare_op=OP.not_equal, fill=1.0,
                base=64, pattern=[[-1, 128]], channel_multiplier=1)
            ones_col = cpool.tile([1, 128], F32, name="ones_col")
            nc.vector.memset(ones_col, 1.0)
            bvr_s = cpool.tile([1, DH_], F32, name="bvr_s")
            nc.sync.dma_start(out=bvr_s, in_=bvr)
            bqc_s = cpool.tile([DK, NH], F32, name="bqc_s")
            nc.sync.dma_start(out=bqc_s, in_=bqc)
            bkc_s = cpool.tile([DK, NH], F32, name="bkc_s")
            nc.sync.dma_start(out=bkc_s, in_=bkc)
            mb_s = None
            if causal:
                mb_s = cpool.tile([DK, 4 * 512], F32, name="mb_s")
                nc.sync.dma_start(
                    out=mb_s.rearrange("p (f c) -> p f c", f=4),
                    in_=mb.rearrange("f p c -> p f c"))

            v_s = vpool.tile([128, n_sc * 4 * DH_], F32R, name="v_s")
            qt_d = dpool.tile([NH, DK, S], F32, name="qt_d")
            kt_d = dpool.tile([NH, DK, S], F32, name="kt_d")

            # ---------------- Phase 1: projections ----------------
            with tc.tile_pool(name="wgt", bufs=1) as wpool, \
                 tc.tile_pool(name="slab", bufs=2) as spool, \
                 tc.tile_pool(name="rope", bufs=1) as rpool, \
                 tc.tile_pool(name="ev", bufs=6) as epool:

                wq_s = wpool.tile([128, n_dc * DH_], F32R, name="wq_s")
                nc.sync.dma_start(
                    out=wq_s.rearrange("p (kc n) -> p kc n", kc=n_dc),
                    in_=wq.bitcast(F32R).rearrange("(kc p) n -> p kc n", p=128))
                wk_s = wpool.tile([128, n_dc * DH_], F32R, name="wk_s")
                nc.sync.dma_start(
                    out=wk_s.rearrange("p (kc n) -> p kc n", kc=n_dc),
                    in_=wk.bitcast(F32R).rearrange("(kc p) n -> p kc n", p=128))
                wv_s = wpool.tile([128, n_dc * DH_], F32R, name="wv_s")
                nc.sync.dma_start(
                    out=wv_s.rearrange("p (kc n) -> p kc n", kc=n_dc),
                    in_=wv.bitcast(F32R).rearrange("(kc p) n -> p kc n", p=128))
                cos_s = rpool.tile([DK, S], F32, name="cos_s")
                nc.sync.dma_start(out=cos_s, in_=cosT)
                sin_s = rpool.tile([DK, S], F32, name="sin_s")
                nc.sync.dma_start(out=sin_s, in_=sinT)

                n_pieces = max(1, n_dc // 4)
                dpp = n_dc // n_pieces

                xTr = xT.bitcast(F32R).rearrange("(kc p) s -> p kc s", p=128)

                for sc in range(n_sc):
                    scs = slice(sc * 512, (sc + 1) * 512)
                    # --- Q/K sweep ---
                    ps_qk = [pp.tile([128, 512], F32, name=f"psqk{t}{h}", tag="ps")
                             for t in range(2) for h in range(NH)]
                    for pc in range(n_pieces):
                        slab = spool.tile([128, dpp * 512], F32R, name="slab")
                        nc.sync.dma_start(
                            out=slab.rearrange("p (i s) -> p i s", i=dpp),
                            in_=xTr[:, pc * dpp:(pc + 1) * dpp, scs])
                        for i in range(dpp):
                            d = pc * dpp + i
                            rhs = slab[:, i * 512:(i + 1) * 512]
                            for h in range(NH):
                                nc.tensor.matmul(
                                    ps_qk[h],
                                    wq_s[:, d * DH_ + h * DK: d * DH_ + (h + 1) * DK],
                                    rhs, start=(d == 0), stop=(d == n_dc - 1))
                                nc.tensor.matmul(
                                    ps_qk[NH + h],
                                    wk_s[:, d * DH_ + h * DK: d * DH_ + (h + 1) * DK],
                                    rhs, start=(d == 0), stop=(d == n_dc - 1))
                    # rope + bias eviction
                    for t in range(2):
                        bcol = bqc_s if t == 0 else bkc_s
                        dst = qt_d if t == 0 else kt_d
                        for h in range(NH):
                            ps = ps_qk[t * NH + h]
                            qsb = epool.tile([128, 512], F32, name="ev_qsb", tag="ev_qsb")
                            nc.vector.tensor_scalar_add(qsb, ps, bcol[:, h:h + 1])
                            rot_ps = pp.tile([128, 512], F32, name="rot_ps", tag="ps")
                            nc.tensor.matmul(rot_ps, rotm, qsb,
                                             start=True, stop=True)
                            tmp = epool.tile([128, 512], F32, name="ev_tmp", tag="ev_tmp")
                            out = epool.tile([128, 512], F32, name="ev_out", tag="ev_out")
                            nc.vector.tensor_mul(out, qsb, cos_s[:, scs])
                            nc.vector.tensor_mul(tmp, rot_ps, sin_s[:, scs])
                            nc.vector.tensor_add(out, out, tmp)
                            nc.sync.dma_start(out=dst[h, :, scs], in_=out)
                    # --- V sweep ---
                    ps_v = [pp.tile([128, DH_], F32, name=f"psv{st}", tag="ps")
                            for st in range(4)]
                    for pc in range(n_pieces):
                        slab = spool.tile([128, dpp * 512], F32R, name="slab")
                        nc.sync.dma_start(
                            out=slab.rearrange("p (i s) -> p i s", i=dpp),
                            in_=xTr[:, pc * dpp:(pc + 1) * dpp, scs])
                        for i in range(dpp):
                            d = pc * dpp + i
                            for st in range(4):
                                nc.tensor.matmul(
                                    ps_v[st],
                                    slab[:, i * 512 + st * 128: i * 512 + (st + 1) * 128],
                                    wv_s[:, d * DH_:(d + 1) * DH_],
                                    start=(d == 0), stop=False)
                    for st in range(4):
                        nc.tensor.matmul(ps_v[st], ones_col, bvr_s,
                                         start=False, stop=True)
                        nc.vector.tensor_copy(
                            v_s[:, (sc * 4 + st) * DH_:(sc * 4 + st + 1) * DH_],
                            ps_v[st])

            if not phase2:
                with tc.tile_pool(name="dump", bufs=4) as dmp:
                    for h in range(NH):
                        for part, dsrc in ((0, qt_d), (NH, kt_d)):
                            dt_ = dmp.tile([128, S], F32, name="dump_t", tag="dump_t")
                            nc.sync.dma_start(out=dt_, in_=dsrc[h])
                            nc.sync.dma_start(out=y[(part + h) * 128:(part + h + 1) * 128, 0:S], in_=dt_)
                    for stt_ in range(4):
                        vt_ = dmp.tile([128, 4 * DH_], F32, name="dump_v", tag="dump_v")
                        nc.vector.tensor_copy(vt_, v_s.bitcast(F32)[:, stt_ * 4 * DH_:(stt_ + 1) * 4 * DH_])
                        nc.sync.dma_start(out=y[(8 + stt_) * 128:(9 + stt_) * 128, 0:4 * DH_], in_=vt_)
            # ---------------- Phase 2: attention ----------------
            if phase2:
              with tc.tile_pool(name="wo_p", bufs=1) as wopool, \
                 tc.tile_pool(name="qt_p", bufs=5) as qtpool, \
                 tc.tile_pool(name="kt_p", bufs=n_sc * NH) as ktpool, \
                 tc.tile_pool(name="pt_p", bufs=17) as ptpool, \
                 tc.tile_pool(name="st_p", bufs=6) as stpool, \
                 tc.tile_pool(name="bb_p", bufs=4) as bbpool, \
                 tc.tile_pool(name="ao_p", bufs=5) as aopool, \
                 tc.tile_pool(name="sc_p", bufs=2) as scpool:

                wo_s = wopool.tile([128, NH * DM], F32R, name="wo_s")
                nc.sync.dma_start(
                    out=wo_s.rearrange("p (h e) -> p h e", h=NH),
                    in_=wo.bitcast(F32R).rearrange("(h p) e -> p h e", p=128))

                for j in range(n_sc):
                    jmax = j if causal else n_sc - 1
                    nch = jmax + 1
                    qt_b = []
                    kt_c = []
                    for h in range(NH):
                        qb = qtpool.tile([128, 512], F32R, name=f"qt_b{h}", tag="qt_b")
                        nc.sync.dma_start(out=qb, in_=qt_d[h, :, j * 512:(j + 1) * 512].bitcast(F32R))
                        qt_b.append(qb)
                        row = []
                        for c in range(nch):
                            kb = ktpool.tile([128, 512], F32R, name=f"kt_c{h}_{c}", tag="kt_c")
                            nc.sync.dma_start(out=kb, in_=kt_d[h, :, c * 512:(c + 1) * 512].bitcast(F32R))
                            row.append(kb)
                        kt_c.append(row)

                    if p2_stage < 1:
                        continue
                    bias_sb = []
                    # ---- pass 1: per-row softmax stats per head ----
                    for h in range(NH):
                        stats_col = stpool.tile([128, 4], F32, name="stats_col",
                                                tag="stats_col")
                        for rl in range(4):
                            pss = []
                            mxs = stpool.tile([128, nch], F32, name="mxs", tag="mxs")
                            for c in range(nch):
                                ps = pp.tile([128, 512], F32, name="ps_s", tag="ps")
                                nc.tensor.matmul(
                                    ps, qt_b[h][:, rl * 128:(rl + 1) * 128],
                                    kt_c[h][c], start=True, stop=True)
                                if causal and c == jmax:
                                    nc.vector.tensor_add(
                                        ps, ps, mb_s[:, rl * 512:(rl + 1) * 512])
                                nc.vector.reduce_max(out=mxs[:, c:c + 1],
                                                     in_=ps, axis=AX.X)
                                pss.append(ps)
                            mx = stpool.tile([128, 1], F32, name="mx", tag="mx")
                            nc.vector.reduce_max(out=mx, in_=mxs, axis=AX.X)
                            negb = stpool.tile([128, 1], F32, name="negb", tag="negb")
                            nc.vector.tensor_scalar_mul(negb, mx, -scale_c)
                            sums = stpool.tile([128, nch], F32, name="sums", tag="sums")
                            scratch = scpool.tile([128, 512], F32, name="scratch",
                                                  tag="scratch")
                            for c in range(nch):
                                nc.scalar.activation(
                                    out=scratch, in_=pss[c], func=ACTF.Exp,
                                    bias=negb, scale=scale_c,
                                    accum_out=sums[:, c:c + 1])
                            sumt = stpool.tile([128, 1], F32, name="sumt", tag="sumt")
                            nc.vector.reduce_sum(out=sumt, in_=sums, axis=AX.X)
                            lns = stpool.tile([128, 1], F32, name="lns", tag="lns")
                            nc.scalar.activation(out=lns, in_=sumt, func=ACTF.Ln)
                            nc.vector.scalar_tensor_tensor(
                                out=stats_col[:, rl:rl + 1], in0=lns,
                                scalar=inv_scale, in1=mx, op0=OP.mult, op1=OP.add)
                        if p2_stage < 2:
                            continue
                        srow_ps = pp.tile([1, 512], F32, name="srow_ps", tag="ps")
                        for rl in range(4):
                            nc.tensor.matmul(
                                srow_ps[0:1, rl * 128:(rl + 1) * 128],
                                stats_col[:, rl:rl + 1], ident,
                                is_transpose=True)
                        srow_sb = stpool.tile([1, 512], F32, name="srow_sb",
                                              tag="srow_sb")
                        nc.vector.tensor_copy(srow_sb, srow_ps[0:1, :])
                        bb = bbpool.tile([128, 512], F32, name="bias_sb", tag="bias_sb")
                        if use_pbcast:
                            nc.gpsimd.partition_broadcast(bb, srow_sb)
                        else:
                            bb_ps = pp.tile([128, 512], F32, name="bb_ps", tag="ps")
                            nc.tensor.matmul(bb_ps, ones_col, srow_sb,
                                             start=True, stop=True)
                            nc.vector.tensor_copy(bb, bb_ps)
                        bias_sb.append(bb)

                    if p2_stage < 3:
                        continue
                    # ---- pass 2 + AV per head ----
                    aoT = []
                    for h in range(NH):
                        nsub = 4 * nch
                        pts = []
                        for t in range(nsub):
                            st_ps = pp.tile([128, 512], F32, name="st_ps", tag="ps")
                            nc.tensor.matmul(
                                st_ps,
                                kt_c[h][t // 4][:, (t % 4) * 128:(t % 4 + 1) * 128],
                                qt_b[h], start=True, stop=True)
                            nc.vector.tensor_sub(st_ps, st_ps, bias_sb[h])
                            pt = ptpool.tile([128, 512], F32R, name="pt", tag="pt")
                            nc.scalar.activation(out=pt, in_=st_ps, func=ACTF.Exp,
                                                 scale=scale_c)
                            p = t - 4 * j
                            if causal and p >= 0 and use_affsel:
                                nc.gpsimd.affine_select(
                                    out=pt, in_=pt, compare_op=OP.is_ge,
                                    fill=0.0, base=-128 * p,
                                    pattern=[[1, 512]], channel_multiplier=-1)
                            pts.append(pt)
                        ao_ps = pp.tile([128, 512], F32, name="ao_ps", tag="ps")
                        for t in range(nsub):
                            nc.tensor.matmul(
                                ao_ps,
                                v_s[:, t * DH_ + h * DK: t * DH_ + (h + 1) * DK],
                                pts[t], start=(t == 0), stop=(t == nsub - 1))
                        ao = aopool.tile([128, 512], F32R, name="aoT", tag="aoT")
                        nc.vector.tensor_copy(ao, ao_ps)
                        aoT.append(ao)

                    if p2_stage < 4:
                        continue
                    # ---- O-projection for this q-block ----
                    for e in range(DM // 512):
                        for sl in range(4):
                            y_ps = pp.tile([128, 512], F32, name="y_ps", tag="ps")
                            for h in range(NH):
                                nc.tensor.matmul(
                                    y_ps, aoT[h][:, sl * 128:(sl + 1) * 128],
                                    wo_s[:, h * DM + e * 512: h * DM + (e + 1) * 512],
                                    start=(h == 0), stop=(h == NH - 1))
                            y_sb = scpool.tile([128, 512], F32, name="y_sb", tag="y_sb")
                            nc.any.tensor_copy(y_sb, y_ps)
                            nc.sync.dma_start(
                                out=y[(j * 4 + sl) * 128:(j * 4 + sl + 1) * 128,
                                      e * 512:(e + 1) * 512],
                                in_=y_sb)

    nc.compile()
    return nc


# ---------------- host side ----------------

def _rope_tables(S_, DK_=DK):
    inv_freq = (1.0 / (10000.0 ** (np.arange(0, DK_, 2, dtype=np.float32) / DK_))
                ).astype(np.float32)
    t = np.arange(S_, dtype=np.float32)
    freqs = np.einsum("i,j->ij", t, inv_freq).astype(np.float32)
    emb = np.concatenate([freqs, freqs], axis=-1)
    return np.cos(emb).astype(np.float32), np.sin(emb).astype(np.float32)


def _mask_tiles_causal():
    mbt = np.zeros((4, 128, 512), dtype=np.float32)
    i = np.arange(128)[:, None]
    c = np.arange(512)[None, :]
    for p in range(4):
        mbt[p] = np.where(c <= i + 128 * p, 0.0, NEG_BIG)
    return mbt


def _core_inputs(x_b, Wq, bq, Wk, bk, Wv, bv, Wo, hg, cosT, sinT, mbt):
    sl = slice(hg * DH, (hg + 1) * DH)
    return {
        "xT": np.ascontiguousarray(x_b.T),
        "wq": np.ascontiguousarray(Wq[:, sl]),
        "wk": np.ascontiguousarray(Wk[:, sl]),
        "wv": np.ascontiguousarray(Wv[:, sl]),
        "wo": np.ascontiguousarray(Wo[sl, :]),
        "bqc": np.ascontiguousarray(bq[sl].reshape(NH, DK).T),
        "bkc": np.ascontiguousarray(bk[sl].reshape(NH, DK).T),
        "bvr": np.ascontiguousarray(bv[sl].reshape(1, DH)),
        "cosT": cosT,
        "sinT": sinT,
        "mb": mbt,
    }


_NC_CACHE = {}


def _get_nc(causal):
    if causal not in _NC_CACHE:
        _NC_CACHE[causal] = build_nc(causal=causal)
    return _NC_CACHE[causal]


def _classify_mask(mask):
    m = np.asarray(mask)
    if np.all(m != 0):
        return "none"
    tril = np.tril(np.ones((S, S), dtype=m.dtype))
    if all(np.array_equal(np.where(m[b, 0] != 0, 1, 0).astype(m.dtype), tril)
           for b in range(m.shape[0])):
        return "causal"
    return "other"


def _numpy_fallback(x, mask, Wq, bq, Wk, bk, Wv, bv, Wo, bo):
    """Correctness fallback for arbitrary masks (host compute)."""
    b_, s_, d_ = x.shape
    q = x @ Wq + bq
    k = x @ Wk + bk
    v = x @ Wv + bv
    q = q.reshape(b_, s_, H, DK).transpose(0, 2, 1, 3)
    k = k.reshape(b_, s_, H, DK).transpose(0, 2, 1, 3)
    v = v.reshape(b_, s_, H, DK).transpose(0, 2, 1, 3)
    cos, sin = _rope_tables(s_)

    def rope(z):
        z1, z2 = z[..., :64], z[..., 64:]
        rot = np.concatenate([-z2, z1], axis=-1)
        return z * cos[None, None] + rot * sin[None, None]
    q, k = rope(q), rope(k)
    scores = np.einsum("bhqd,bhkd->bhqk", q, k) / np.sqrt(np.float32(DK))
    scores = np.where(mask == 0, -np.inf, scores)
    scores = scores - scores.max(axis=-1, keepdims=True)
    attn = np.exp(scores)
    attn = attn / attn.sum(axis=-1, keepdims=True)
    out = np.einsum("bhqk,bhkd->bhqd", attn, v)
    out = out.transpose(0, 2, 1, 3).reshape(b_, s_, d_)
    return (out @ Wo + bo).astype(np.float32)


def run_cores(inputs, causal, trace=False, tmpdir=None):
    """Build in_maps, run the SPMD kernel, return (results, BassKernelResults)."""
    x = np.asarray(inputs["x"], dtype=np.float32)
    cos, sin = _rope_tables(S)
    cosT = np.ascontiguousarray(cos.T)
    sinT = np.ascontiguousarray(sin.T)
    mbt = _mask_tiles_causal()
    in_maps = []
    for c in range(N_CORES):
        b, hg = divmod(c, N_CORES // B)
        in_maps.append(_core_inputs(
            x[b], inputs["Wq"], inputs["bq"], inputs["Wk"], inputs["bk"],
            inputs["Wv"], inputs["bv"], inputs["Wo"], hg, cosT, sinT, mbt))
    nc = _get_nc(causal)
    res = run_bass_kernel_spmd(nc, in_maps, list(range(N_CORES)), trace=trace, tmpdir=tmpdir)
    return res


def kernel(**inputs):
    mask_kind = _classify_mask(inputs["mask"])
    if mask_kind == "other":
        return _numpy_fallback(
            np.asarray(inputs["x"], np.float32), np.asarray(inputs["mask"]),
            np.asarray(inputs["Wq"], np.float32), np.asarray(inputs["bq"], np.float32),
            np.asarray(inputs["Wk"], np.float32), np.asarray(inputs["bk"], np.float32),
            np.asarray(inputs["Wv"], np.float32), np.asarray(inputs["bv"], np.float32),
            np.asarray(inputs["Wo"], np.float32), np.asarray(inputs["bo"], np.float32))
    res = run_cores(inputs, causal=(mask_kind == "causal"))
    ngroups = N_CORES // B
    bo = np.asarray(inputs["bo"], dtype=np.float32)
    out = np.empty((B, S, D), dtype=np.float32)
    for b in range(B):
        acc = res.results[b * ngroups]["y"].astype(np.float32)
        for g in range(1, ngroups):
            acc = acc + res.results[b * ngroups + g]["y"]
        out[b] = acc + bo
    return out


# revision 9
# speedup vs baseline: 1.1891x; 1.1891x over previous
"""Multi-head attention (RoPE + causal mask) Trainium2 kernel, 8-core SPMD.

Sharding: 8 cores = 2 batches x 4 head-groups (4 heads of dk=128 each).
Each core computes q/k/v projections for its head-group, attention, and a
partial output projection; the host sums the 4 head-group partials per batch.

Per-core device program (Bass/Tile):
  - qT, kT computed transposed [dk, S] with RoPE fused at PSUM eviction
    (rotate-half via a +-1 permutation matmul on the PE), spilled to DRAM.
  - v computed natural [S, dv-group], resident in SBUF.
  - pass 1 [s_q part, s_k free]: causal-mask add + row-max only (DVE).
  - pass 2 transposed [s_k part, s_q free]: row max subtracted by a rank-1
    ones x (-max) matmul accumulated into the scores PSUM, then
    P~ = exp(scale*(s-mx)) straight to SBUF (ACT); masked region zeroed by
    GpSimd affine_select. Softmax denominators = ones^T @ P~ accumulated on
    the PE; normalization folds into the aoT PSUM eviction multiply, which
    is exact because the sums are of the same rounded P~ the AV matmul uses.
  - AV on PE: aoT[dv, s_q] += V^T_tile @ P~^T_tile; O-projection accumulates
    the 4 heads in PSUM; y tiles DMA'd out.
  - fp32 data flows through matmuls as float32r (full-rate fp32 path,
    fp32 PSUM accumulation).
"""

import numpy as np

import concourse.bacc as bacc
import concourse.mybir as mybir
from concourse.tile import TileContext
from concourse.masks import make_identity
from concourse.bass_utils import run_bass_kernel_spmd

F32 = mybir.dt.float32
F32R = mybir.dt.float32r
AX = mybir.AxisListType
OP = mybir.AluOpType
ACTF = mybir.ActivationFunctionType

B, S, D, H = 2, 2048, 2048, 16
DK = 128
NH = 4                      # heads per core
DH = NH * DK                # head-group width
N_CORES = 8
NEG_BIG = -1.0e9


def build_nc(causal=True, S=S, DM=D, NH=NH):
    DH_ = NH * DK
    n_dc = DM // DK
    n_sc = S // 512
    scale_c = 1.0 / float(np.sqrt(DK))

    nc = bacc.Bacc("TRN2", target_bir_lowering=False, debug=False,
                   enable_asserts=False, num_devices=N_CORES)

    xT = nc.dram_tensor("xT", (DM, S), F32, kind="ExternalInput").ap()
    wq = nc.dram_tensor("wq", (DM, DH_), F32, kind="ExternalInput").ap()
    wk = nc.dram_tensor("wk", (DM, DH_), F32, kind="ExternalInput").ap()
    wv = nc.dram_tensor("wv", (DM, DH_), F32, kind="ExternalInput").ap()
    wo = nc.dram_tensor("wo", (DH_, DM), F32, kind="ExternalInput").ap()
    bqc = nc.dram_tensor("bqc", (DK, NH), F32, kind="ExternalInput").ap()
    bkc = nc.dram_tensor("bkc", (DK, NH), F32, kind="ExternalInput").ap()
    bvr = nc.dram_tensor("bvr", (1, DH_), F32, kind="ExternalInput").ap()
    cosT = nc.dram_tensor("cosT", (DK, S), F32, kind="ExternalInput").ap()
    sinT = nc.dram_tensor("sinT", (DK, S), F32, kind="ExternalInput").ap()
    ones_in = nc.dram_tensor("ones_in", (DK, 2), F32, kind="ExternalInput").ap()
    mb = nc.dram_tensor("mb", (4, DK, 512), F32, kind="ExternalInput").ap()
    y = nc.dram_tensor("y", (S, DM), F32, kind="ExternalOutput").ap()

    with TileContext(nc) as tc:
        with tc.tile_pool(name="const", bufs=1) as cpool, \
             tc.tile_pool(name="dram", bufs=1, space="DRAM") as dpool, \
             tc.tile_pool(name="vres", bufs=1) as vpool, \
             tc.tile_pool(name="psum", bufs=8, space="PSUM") as pp:

            ident = cpool.tile([128, 128], F32, name="ident")
            make_identity(nc, ident)
            # rotate-half matrix: rotm[d, m] = -1 if d==m+64, +1 if d==m-64
            rotm = cpool.tile([128, 128], F32, name="rotm")
            nc.gpsimd.memset(rotm, 0.0)
            nc.gpsimd.affine_select(
                out=rotm, in_=rotm, compare_op=OP.not_equal, fill=-1.0,
                base=-64, pattern=[[-1, 128]], channel_multiplier=1)
            nc.gpsimd.affine_select(
                out=rotm, in_=rotm, compare_op=OP.not_equal, fill=1.0,
                base=64, pattern=[[-1, 128]], channel_multiplier=1)
            ones_col = cpool.tile([1, 128], F32, name="ones_col")
            nc.vector.memset(ones_col, 1.0)
            # f32r ones: [128,1] column (sum-matmul lhsT), [1,128] row (bias)
            onesr = cpool.tile([DK, 2], F32R, name="onesr")
            nc.sync.dma_start(out=onesr, in_=ones_in.bitcast(F32R))
            onesr_row = cpool.tile([1, 128], F32R, name="onesr_row")
            nc.sync.dma_start(
                out=onesr_row,
                in_=ones_in.bitcast(F32R)[:, 0:1].rearrange("p o -> o p"))
            bvr_s = cpool.tile([1, DH_], F32, name="bvr_s")
            nc.sync.dma_start(out=bvr_s, in_=bvr)
            bqc_s = cpool.tile([DK, NH], F32, name="bqc_s")
            nc.sync.dma_start(out=bqc_s, in_=bqc)
            bkc_s = cpool.tile([DK, NH], F32, name="bkc_s")
            nc.sync.dma_start(out=bkc_s, in_=bkc)
            mb_s = None
            if causal:
                mb_s = cpool.tile([DK, 4 * 512], F32, name="mb_s")
                nc.sync.dma_start(
                    out=mb_s.rearrange("p (f c) -> p f c", f=4),
                    in_=mb.rearrange("f p c -> p f c"))

            v_s = vpool.tile([128, n_sc * 4 * DH_], F32R, name="v_s")
            qt_d = dpool.tile([NH, DK, S], F32, name="qt_d")
            kt_d = dpool.tile([NH, DK, S], F32, name="kt_d")

            # ---------------- Phase 1: projections ----------------
            with tc.tile_pool(name="wgt", bufs=1) as wpool, \
                 tc.tile_pool(name="slab", bufs=3) as spool, \
                 tc.tile_pool(name="rope", bufs=1) as rpool, \
                 tc.tile_pool(name="ev", bufs=4) as epool:

                wq_s = wpool.tile([128, n_dc * DH_], F32R, name="wq_s")
                nc.sync.dma_start(
                    out=wq_s.rearrange("p (kc n) -> p kc n", kc=n_dc),
                    in_=wq.bitcast(F32R).rearrange("(kc p) n -> p kc n", p=128))
                wk_s = wpool.tile([128, n_dc * DH_], F32R, name="wk_s")
                nc.sync.dma_start(
                    out=wk_s.rearrange("p (kc n) -> p kc n", kc=n_dc),
                    in_=wk.bitcast(F32R).rearrange("(kc p) n -> p kc n", p=128))
                wv_s = wpool.tile([128, n_dc * DH_], F32R, name="wv_s")
                nc.sync.dma_start(
                    out=wv_s.rearrange("p (kc n) -> p kc n", kc=n_dc),
                    in_=wv.bitcast(F32R).rearrange("(kc p) n -> p kc n", p=128))
                cos_s = rpool.tile([DK, S], F32, name="cos_s")
                nc.sync.dma_start(out=cos_s, in_=cosT)
                sin_s = rpool.tile([DK, S], F32, name="sin_s")
                nc.sync.dma_start(out=sin_s, in_=sinT)

                n_pieces = max(1, n_dc // 4)
                dpp = n_dc // n_pieces

                xTr = xT.bitcast(F32R).rearrange("(kc p) s -> p kc s", p=128)

                def evict_rope(ps, bcol, h, dst, scs):
                    """RoPE + bias eviction of one qT/kT psum tile."""
                    qsb = epool.tile([128, 512], F32, name="ev_qsb", tag="ev_qsb")
                    nc.vector.tensor_scalar_add(qsb, ps, bcol[:, h:h + 1])
                    rot_ps = pp.tile([128, 512], F32, name="rot_ps", tag="ps")
                    nc.tensor.matmul(rot_ps, rotm, qsb, start=True, stop=True)
                    tmp = epool.tile([128, 512], F32, name="ev_tmp", tag="ev_tmp")
                    out = epool.tile([128, 512], F32, name="ev_out", tag="ev_out")
                    nc.vector.tensor_mul(out, qsb, cos_s[:, scs])
                    nc.vector.tensor_mul(tmp, rot_ps, sin_s[:, scs])
                    nc.vector.tensor_add(out, out, tmp)
                    nc.sync.dma_start(out=dst[h, :, scs], in_=out)

                for sc in range(n_sc):
                    scs = slice(sc * 512, (sc + 1) * 512)
                    # --- Q/K sweep ---
                    ps_qk = [pp.tile([128, 512], F32, name=f"psqk{t}{h}", tag="ps")
                             for t in range(2) for h in range(NH)]
                    for pc in range(n_pieces):
                        slab = spool.tile([128, dpp * 512], F32R, name="slab")
                        nc.sync.dma_start(
                            out=slab.rearrange("p (i s) -> p i s", i=dpp),
                            in_=xTr[:, pc * dpp:(pc + 1) * dpp, scs])
                        for i in range(dpp):
                            d = pc * dpp + i
                            rhs = slab[:, i * 512:(i + 1) * 512]
                            for h in range(NH):
                                nc.tensor.matmul(
                                    ps_qk[h],
                                    wq_s[:, d * DH_ + h * DK: d * DH_ + (h + 1) * DK],
                                    rhs, start=(d == 0), stop=(d == n_dc - 1))
                                nc.tensor.matmul(
                                    ps_qk[NH + h],
                                    wk_s[:, d * DH_ + h * DK: d * DH_ + (h + 1) * DK],
                                    rhs, start=(d == 0), stop=(d == n_dc - 1))
                    # evict K first (frees PSUM banks for the V sweep; Q
                    # evictions then overlap the V matmuls)
                    for h in range(NH):
                        evict_rope(ps_qk[NH + h], bkc_s, h, kt_d, scs)
                    # --- V sweep ---
                    ps_v = [pp.tile([128, DH_], F32, name=f"psv{st}", tag="ps")
                            for st in range(4)]
                    for pc in range(n_pieces):
                        slab = spool.tile([128, dpp * 512], F32R, name="slab")
                        nc.sync.dma_start(
                            out=slab.rearrange("p (i s) -> p i s", i=dpp),
                            in_=xTr[:, pc * dpp:(pc + 1) * dpp, scs])
                        for i in range(dpp):
                            d = pc * dpp + i
                            for st in range(4):
                                nc.tensor.matmul(
                                    ps_v[st],
                                    slab[:, i * 512 + st * 128: i * 512 + (st + 1) * 128],
                                    wv_s[:, d * DH_:(d + 1) * DH_],
                                    start=(d == 0), stop=False)
                    for h in range(NH):
                        evict_rope(ps_qk[h], bqc_s, h, qt_d, scs)
                    for st in range(4):
                        nc.tensor.matmul(ps_v[st], ones_col, bvr_s,
                                         start=False, stop=True)
                        nc.vector.tensor_copy(
                            v_s[:, (sc * 4 + st) * DH_:(sc * 4 + st + 1) * DH_],
                            ps_v[st])

            # ---------------- Phase 2: attention ----------------
            with tc.tile_pool(name="wo_p", bufs=1) as wopool, \
                 tc.tile_pool(name="qt_p", bufs=5) as qtpool, \
                 tc.tile_pool(name="kt_p", bufs=n_sc * NH) as ktpool, \
                 tc.tile_pool(name="pt_p", bufs=17) as ptpool, \
                 tc.tile_pool(name="st_p", bufs=6) as stpool, \
                 tc.tile_pool(name="bb_p", bufs=4) as bbpool, \
                 tc.tile_pool(name="ao_p", bufs=5) as aopool, \
                 tc.tile_pool(name="sc_p", bufs=3) as scpool:

                wo_s = wopool.tile([128, NH * DM], F32R, name="wo_s")
                nc.sync.dma_start(
                    out=wo_s.rearrange("p (h e) -> p h e", h=NH),
                    in_=wo.bitcast(F32R).rearrange("(h p) e -> p h e", p=128))

                for j in range(n_sc):
                    jmax = j if causal else n_sc - 1
                    nch = jmax + 1
                    qt_b = []
                    kt_c = []
                    for h in range(NH):
                        qb = qtpool.tile([128, 512], F32R, name=f"qt_b{h}", tag="qt_b")
                        nc.sync.dma_start(
                            out=qb, in_=qt_d[h, :, j * 512:(j + 1) * 512].bitcast(F32R))
                        qt_b.append(qb)
                        row = []
                        for c in range(nch):
                            kb = ktpool.tile([128, 512], F32R, name=f"kt_c{h}_{c}",
                                             tag="kt_c")
                            nc.sync.dma_start(
                                out=kb,
                                in_=kt_d[h, :, c * 512:(c + 1) * 512].bitcast(F32R))
                            row.append(kb)
                        kt_c.append(row)

                    # ---- pass 1: row max per head (no exp) ----
                    nmx_cols = []
                    for h in range(NH):
                        nmx = stpool.tile([128, 4], F32, name="nmx", tag="nmx")
                        for rl in range(4):
                            mxs = stpool.tile([128, nch], F32, name="mxs", tag="mxs")
                            for c in range(nch):
                                ps = pp.tile([128, 512], F32, name="ps_s", tag="ps")
                                nc.tensor.matmul(
                                    ps, qt_b[h][:, rl * 128:(rl + 1) * 128],
                                    kt_c[h][c], start=True, stop=True)
                                if causal and c == jmax:
                                    nc.vector.tensor_add(
                                        ps, ps, mb_s[:, rl * 512:(rl + 1) * 512])
                                nc.vector.reduce_max(out=mxs[:, c:c + 1],
                                                     in_=ps, axis=AX.X)
                            nc.vector.reduce_max(out=nmx[:, rl:rl + 1],
                                                 in_=mxs, axis=AX.X)
                        nc.vector.tensor_scalar_mul(nmx, nmx, -1.0)
                        nmx_cols.append(nmx)

                    # ---- -max rows for the rank-1 bias matmuls ----
                    nmx_rows = []
                    for h in range(NH):
                        srow_ps = pp.tile([1, 512], F32, name="srow_ps", tag="ps")
                        for rl in range(4):
                            nc.tensor.matmul(
                                srow_ps[0:1, rl * 128:(rl + 1) * 128],
                                nmx_cols[h][:, rl:rl + 1], ident,
                                is_transpose=True)
                        srow = stpool.tile([1, 512], F32R, name="srow", tag="srow")
                        nc.vector.tensor_copy(srow, srow_ps[0:1, :])
                        nmx_rows.append(srow)

                    # ---- pass 2: P~, AV, sums per head ----
                    aoT = []
                    for h in range(NH):
                        nsub = 4 * nch
                        ao_ps = pp.tile([128, 512], F32, name="ao_ps", tag="ps")
                        sum_ps = pp.tile([1, 512], F32, name="sum_ps", tag="ps")
                        for t in range(nsub):
                            st_ps = pp.tile([128, 512], F32, name="st_ps", tag="ps")
                            nc.tensor.matmul(
                                st_ps,
                                kt_c[h][t // 4][:, (t % 4) * 128:(t % 4 + 1) * 128],
                                qt_b[h], start=True, stop=False)
                            nc.tensor.matmul(
                                st_ps, onesr_row, nmx_rows[h],
                                start=False, stop=True)
                            pt = ptpool.tile([128, 512], F32R, name="pt", tag="pt")
                            nc.scalar.activation(out=pt, in_=st_ps, func=ACTF.Exp,
                                                 scale=scale_c)
                            p = t - 4 * j
                            if causal and p >= 0:
                                nc.gpsimd.affine_select(
                                    out=pt, in_=pt, compare_op=OP.is_ge,
                                    fill=0.0, base=-128 * p,
                                    pattern=[[1, 512]], channel_multiplier=-1)
                            nc.tensor.matmul(
                                ao_ps, v_s[:, t * DH_ + h * DK: t * DH_ + (h + 1) * DK],
                                pt, start=(t == 0), stop=(t == nsub - 1))
                            nc.tensor.matmul(
                                sum_ps, onesr[:, 0:1], pt,
                                start=(t == 0), stop=(t == nsub - 1))
                        rsum = stpool.tile([1, 512], F32, name="rsum", tag="rsum")
                        nc.vector.reciprocal(rsum, sum_ps[0:1, :])
                        bb = bbpool.tile([128, 512], F32, name="bb", tag="bb")
                        nc.gpsimd.partition_broadcast(bb, rsum)
                        ao = aopool.tile([128, 512], F32R, name="aoT", tag="aoT")
                        nc.vector.tensor_mul(ao, ao_ps, bb)
                        aoT.append(ao)

                    # ---- O-projection for this q-block ----
                    for e in range(DM // 512):
                        for sl in range(4):
                            y_ps = pp.tile([128, 512], F32, name="y_ps", tag="ps")
                            for h in range(NH):
                                nc.tensor.matmul(
                                    y_ps, aoT[h][:, sl * 128:(sl + 1) * 128],
                                    wo_s[:, h * DM + e * 512: h * DM + (e + 1) * 512],
                                    start=(h == 0), stop=(h == NH - 1))
                            y_sb = scpool.tile([128, 512], F32, name="y_sb", tag="y_sb")
                            nc.scalar.activation(out=y_sb, in_=y_ps, func=ACTF.Copy)
                            nc.sync.dma_start(
                                out=y[(j * 4 + sl) * 128:(j * 4 + sl + 1) * 128,
                                      e * 512:(e + 1) * 512],
                                in_=y_sb)

    nc.compile()
    return nc


# ---------------- host side ----------------

def _rope_tables(S_, DK_=DK):
    inv_freq = (1.0 / (10000.0 ** (np.arange(0, DK_, 2, dtype=np.float32) / DK_))
                ).astype(np.float32)
    t = np.arange(S_, dtype=np.float32)
    freqs = np.einsum("i,j->ij", t, inv_freq).astype(np.float32)
    emb = np.concatenate([freqs, freqs], axis=-1)
    return np.cos(emb).astype(np.float32), np.sin(emb).astype(np.float32)


def _mask_tiles_causal():
    mbt = np.zeros((4, 128, 512), dtype=np.float32)
    i = np.arange(128)[:, None]
    c = np.arange(512)[None, :]
    for p in range(4):
        mbt[p] = np.where(c <= i + 128 * p, 0.0, NEG_BIG)
    return mbt


def _core_inputs(x_b, Wq, bq, Wk, bk, Wv, bv, Wo, hg, cosT, sinT, mbt):
    sl = slice(hg * DH, (hg + 1) * DH)
    return {
        "xT": np.ascontiguousarray(x_b.T),
        "wq": np.ascontiguousarray(Wq[:, sl]),
        "wk": np.ascontiguousarray(Wk[:, sl]),
        "wv": np.ascontiguousarray(Wv[:, sl]),
        "wo": np.ascontiguousarray(Wo[sl, :]),
        "bqc": np.ascontiguousarray(bq[sl].reshape(NH, DK).T),
        "bkc": np.ascontiguousarray(bk[sl].reshape(NH, DK).T),
        "bvr": np.ascontiguousarray(bv[sl].reshape(1, DH)),
        "cosT": cosT,
        "sinT": sinT,
        "ones_in": np.ones((DK, 2), dtype=np.float32),
        "mb": mbt,
    }


_NC_CACHE = {}


def _get_nc(causal):
    if causal not in _NC_CACHE:
        _NC_CACHE[causal] = build_nc(causal=causal)
    return _NC_CACHE[causal]


def _classify_mask(mask):
    m = np.asarray(mask)
    if np.all(m != 0):
        return "none"
    tril = np.tril(np.ones((S, S), dtype=m.dtype))
    if all(np.array_equal(np.where(m[b, 0] != 0, 1, 0).astype(m.dtype), tril)
           for b in range(m.shape[0])):
        return "causal"
    return "other"


def _numpy_fallback(x, mask, Wq, bq, Wk, bk, Wv, bv, Wo, bo):
    """Correctness fallback for arbitrary masks (host compute)."""
    b_, s_, d_ = x.shape
    q = x @ Wq + bq
    k = x @ Wk + bk
    v = x @ Wv + bv
    q = q.reshape(b_, s_, H, DK).transpose(0, 2, 1, 3)
    k = k.reshape(b_, s_, H, DK).transpose(0, 2, 1, 3)
    v = v.reshape(b_, s_, H, DK).transpose(0, 2, 1, 3)
    cos, sin = _rope_tables(s_)

    def rope(z):
        z1, z2 = z[..., :64], z[..., 64:]
        rot = np.concatenate([-z2, z1], axis=-1)
        return z * cos[None, None] + rot * sin[None, None]
    q, k = rope(q), rope(k)
    scores = np.einsum("bhqd,bhkd->bhqk", q, k) / np.sqrt(np.float32(DK))
    scores = np.where(mask == 0, -np.inf, scores)
    scores = scores - scores.max(axis=-1, keepdims=True)
    attn = np.exp(scores)
    attn = attn / attn.sum(axis=-1, keepdims=True)
    out = np.einsum("bhqk,bhkd->bhqd", attn, v)
    out = out.transpose(0, 2, 1, 3).reshape(b_, s_, d_)
    return (out @ Wo + bo).astype(np.float32)


def run_cores(inputs, causal, trace=False, tmpdir=None):
    """Build in_maps, run the SPMD kernel, return BassKernelResults."""
    x = np.asarray(inputs["x"], dtype=np.float32)
    cos, sin = _rope_tables(S)
    cosT = np.ascontiguousarray(cos.T)
    sinT = np.ascontiguousarray(sin.T)
    mbt = _mask_tiles_causal()
    in_maps = []
    for c in range(N_CORES):
        b, hg = divmod(c, N_CORES // B)
        in_maps.append(_core_inputs(
            x[b], inputs["Wq"], inputs["bq"], inputs["Wk"], inputs["bk"],
            inputs["Wv"], inputs["bv"], inputs["Wo"], hg, cosT, sinT, mbt))
    nc = _get_nc(causal)
    res = run_bass_kernel_spmd(nc, in_maps, list(range(N_CORES)), trace=trace,
                               tmpdir=tmpdir)
    return res


def kernel(**inputs):
    mask_kind = _classify_mask(inputs["mask"])
    if mask_kind == "other":
        return _numpy_fallback(
            np.asarray(inputs["x"], np.float32), np.asarray(inputs["mask"]),
            np.asarray(inputs["Wq"], np.float32), np.asarray(inputs["bq"], np.float32),
            np.asarray(inputs["Wk"], np.float32), np.asarray(inputs["bk"], np.float32),
            np.asarray(inputs["Wv"], np.float32), np.asarray(inputs["bv"], np.float32),
            np.asarray(inputs["Wo"], np.float32), np.asarray(inputs["bo"], np.float32))
    res = run_cores(inputs, causal=(mask_kind == "causal"))
    ngroups = N_CORES // B
    bo = np.asarray(inputs["bo"], dtype=np.float32)
    out = np.empty((B, S, D), dtype=np.float32)
    for b in range(B):
        acc = res.results[b * ngroups]["y"].astype(np.float32)
        for g in range(1, ngroups):
            acc = acc + res.results[b * ngroups + g]["y"]
        out[b] = acc + bo
    return out
